# revision 25
# baseline (speedup 1.0000x reference)
"""Trainium2 Bass kernel for nn_DIYloss_1709396984424.

Loss: for binary labels, mean over (one, zero) pairs of (1 + p[l] - p[k])^2
where p = sigmoid(pred_Y). The L^2 pairwise sum has an exact closed form in
O(L) masked reductions:

    pair_sum = n1*Q2 - 2*s1*Q1 + n0*s2
      s1 = sum(m1*p), s2 = sum(m1*p^2)
      s0_1 = sum(p) - s1, s0_2 = sum(p^2) - s2, n0 = L - n1
      Q1 = n0 + s0_1,  Q2 = n0 + 2*s0_1 + s0_2

    loss = pair_sum / max(n1*n0, 1) + [n1 == 0] * mean(p^2)
    (pair_sum is exactly 0 when n1 == 0, so the blend needs no subtraction)

Each of the 8 cores receives the full (replicated) input and computes the
full scalar on-device; core 0's output is returned. The two inputs are
packed host-side into one [128,128] f32 buffer (int32 labels bitcast into
the second half) so a single DMA brings everything in. Per-core work: one
32 KiB DMA, ACT sigmoid/squares with fused row-sums, a few DVE ops, one
tiny PE matmul for the partition-axis sum, and a short scalar epilogue.
"""

import numpy as np

try:
    import concourse.bass as bass  # noqa: F401
except ImportError:  # pragma: no cover - grading env should have it on path
    import sys

    sys.path.insert(0, "/opt/trn_rl_repo")
    import concourse.bass as bass  # noqa: F401

import concourse.tile as tile
from concourse import bacc, mybir
from concourse.bass_utils import run_bass_kernel_spmd

L = 8192
P = 128
F = L // P  # 64
N_CORES = 8

_f32 = mybir.dt.float32
_i32 = mybir.dt.int32
_Alu = mybir.AluOpType
_Act = mybir.ActivationFunctionType

_built = None


def _build():
    nc = bacc.Bacc(
        "TRN2", debug=False, target_bir_lowering=False, num_devices=N_CORES
    )
    # cols 0:F = pred_Y (f32), cols F:2F = true_Y (int32 bitcast to f32)
    xin_d = nc.dram_tensor("xin", [P, 2 * F], _f32, kind="ExternalInput")
    out_d = nc.dram_tensor("out", [1, 1], _f32, kind="ExternalOutput")

    with tile.TileContext(nc) as tc:
        with (
            tc.tile_pool(name="sbuf", bufs=1) as pool,
            tc.tile_pool(name="psum", bufs=1, space="PSUM") as psum,
        ):
            xt = pool.tile([P, 2 * F], _f32)
            nc.sync.dma_start(xt[:], xin_d[:])
            pred_v = xt[:, 0:F]
            true_v = xt[:, F : 2 * F].bitcast(_i32)

            p = pool.tile([P, F], _f32)
            p2 = pool.tile([P, F], _f32)
            m1 = pool.tile([P, F], _f32)
            mp = pool.tile([P, F], _f32)
            mp2 = pool.tile([P, F], _f32)
            stats = pool.tile([P, 8], _f32)
            ones = pool.tile([P, 1], _f32)

            # stats columns (per-partition row sums):
            # 0: sum(p^2)  1: n1  2: s1=sum(m1*p)  3: s2=sum((m1*p)^2)
            # 4: s0_1=sum(p-m1*p)  5: s0_2=sum(p^2-(m1*p)^2)
            # s0_* are summed from element-wise differences (not total minus
            # total) so they are exactly 0 when the mask is degenerate.
            # Every DVE producer op carries its row-sum via accum_out; ACT
            # only does the sigmoid.
            # NOTE: only mybir-level ops here; raw bass_isa opcodes (e.g.
            # tensor_tensor_reduce) crash the neuronx-cc/PJRT execution path.
            m0p = pool.tile([P, F], _f32)
            m0p2 = pool.tile([P, F], _f32)
            nc.scalar.activation(p[:], pred_v, _Act.Sigmoid)
            # m1 = float(true), n1 = rowsum(m1)
            nc.vector.tensor_copy(m1[:], true_v)  # int32 -> f32 cast, values 0/1
            nc.vector.tensor_reduce(
                stats[:, 1:2], m1[:], axis=mybir.AxisListType.X, op=_Alu.add
            )
            # mp = m1*p, s1 = rowsum(mp)
            nc.vector.scalar_tensor_tensor(
                out=mp[:], in0=m1[:], scalar=1.0, in1=p[:],
                op0=_Alu.mult, op1=_Alu.mult, accum_out=stats[:, 2:3],
            )
            # p2 = p*p, t2 = rowsum(p2)
            nc.vector.scalar_tensor_tensor(
                out=p2[:], in0=p[:], scalar=1.0, in1=p[:],
                op0=_Alu.mult, op1=_Alu.mult, accum_out=stats[:, 0:1],
            )
            # mp2 = mp*mp, s2 = rowsum(mp2); m1 is 0/1 so (m1*p)^2 == m1*p^2
            nc.vector.scalar_tensor_tensor(
                out=mp2[:], in0=mp[:], scalar=1.0, in1=mp[:],
                op0=_Alu.mult, op1=_Alu.mult, accum_out=stats[:, 3:4],
            )
            nc.vector.scalar_tensor_tensor(
                out=m0p[:], in0=mp[:], scalar=-1.0, in1=p[:],
                op0=_Alu.mult, op1=_Alu.add, accum_out=stats[:, 4:5],
            )
            nc.vector.scalar_tensor_tensor(
                out=m0p2[:], in0=mp2[:], scalar=-1.0, in1=p2[:],
                op0=_Alu.mult, op1=_Alu.add, accum_out=stats[:, 5:6],
            )

            # Partition-axis reduction: ones^T [128,1] @ stats[:, 0:6] -> [1,6]
            nc.vector.memset(ones[:], 1.0)
            acc = psum.tile([1, 8], _f32)
            nc.tensor.matmul(
                acc[0:1, 0:6], ones[:], stats[:, 0:6], start=True, stop=True
            )
            # HW rule NCC_IBVF027: at most one non-scalar PSUM operand per
            # instruction — land the totals in SBUF once, then stay in SBUF.
            r = pool.tile([1, 8], _f32)
            nc.vector.tensor_copy(r[0:1, 0:6], acc[0:1, 0:6])
            t2 = r[0:1, 0:1]  # sum(p^2)
            n1 = r[0:1, 1:2]
            s1 = r[0:1, 2:3]
            s2 = r[0:1, 3:4]
            s0_1 = r[0:1, 4:5]
            s0_2 = r[0:1, 5:6]

            w = pool.tile([1, 16], _f32)

            def c(i):
                return w[0:1, i : i + 1]

            # ACT (idle by now) computes the two totals-only affine terms.
            nc.scalar.activation(
                c(2), n1, _Act.Copy, bias=float(L), scale=-1.0
            )  # n0 = L - n1
            nc.scalar.activation(
                c(12), t2, _Act.Copy, bias=0.0, scale=1.0 / L
            )  # zero_loss = mean(p^2)
            nc.vector.tensor_add(c(3), c(2), s0_1)  # Q1 = n0 + s0_1
            nc.vector.tensor_add(c(4), c(3), s0_1)  # n0 + 2*s0_1
            nc.vector.tensor_add(c(5), c(4), s0_2)  # Q2
            # G = (s1*2)*Q1
            nc.vector.scalar_tensor_tensor(
                out=c(6), in0=s1, scalar=2.0, in1=c(3), op0=_Alu.mult, op1=_Alu.mult
            )
            # H = (n0*s2) - G
            nc.vector.scalar_tensor_tensor(
                out=c(7), in0=c(2), scalar=s2, in1=c(6),
                op0=_Alu.mult, op1=_Alu.subtract,
            )
            # pair_sum = (n1*Q2) + H
            nc.vector.scalar_tensor_tensor(
                out=c(8), in0=n1, scalar=c(5), in1=c(7),
                op0=_Alu.mult, op1=_Alu.add,
            )
            # denom = max(n1*n0, 1); integers so equals where(n1*n0>0, n1*n0, 1)
            nc.vector.scalar_tensor_tensor(
                out=c(9), in0=c(2), scalar=n1, in1=ones[0:1, 0:1],
                op0=_Alu.mult, op1=_Alu.max,
            )
            nc.vector.reciprocal(c(10), c(9))
            nc.vector.tensor_mul(c(11), c(8), c(10))  # pair_loss
            # flag = [n1 == 0]
            nc.vector.tensor_scalar(
                out=c(13), in0=n1, scalar1=0.0, scalar2=None, op0=_Alu.is_equal
            )
            # out = pair_loss + flag*zero_loss (pair_loss == 0 exactly when n1==0)
            nc.vector.scalar_tensor_tensor(
                out=c(14), in0=c(13), scalar=c(12), in1=c(11),
                op0=_Alu.mult, op1=_Alu.add,
            )

            nc.sync.dma_start(out_d[:], c(14))

    nc.compile()
    return nc


def _pack(pred_Y, true_Y):
    xin = np.empty((P, 2 * F), dtype=np.float32)
    xin[:, 0:F] = np.ascontiguousarray(pred_Y, dtype=np.float32).reshape(P, F)
    xin[:, F : 2 * F] = (
        np.ascontiguousarray(true_Y, dtype=np.int32).reshape(P, F).view(np.float32)
    )
    return xin


def _run(pred_Y, true_Y, **hw_kwargs):
    global _built
    if _built is None:
        _built = _build()
    in_map = {"xin": _pack(pred_Y, true_Y)}
    res = run_bass_kernel_spmd(
        _built, [in_map] * N_CORES, list(range(N_CORES)), **hw_kwargs
    )
    out = np.asarray(res.results[0]["out"], dtype=np.float32).reshape(())
    return out, res


def kernel(pred_Y, true_Y):
    out, _ = _run(pred_Y, true_Y)
    return out
